# revision 17
# baseline (speedup 1.0000x reference)
"""Trainium2 Bass kernel for nn_CICDM — pair-feature reformulation, v7.

Math: the Choquet integral C[n,b] is linear in shared features
  F = [R (435 pair hinges), sel (30), U (1024 per-exercise triple mins)]
  R[p=(i<j)] = relu(sel_i - sel_j)
  U[n] = min(R[p02(n)], R[p12(n)]) = relu(min(d02, d12))
so layer-1 of the MLP folds the whole per-exercise coefficient structure
into a host-precomputed W1F = w1 @ Gamma^T:  z1 = W1F @ F + b1.
The device never materializes C.

v7: R and U feature blocks both run as fp8-e4m3 DoubleRow matmuls (only
the 30-row sel chunk stays bf16; host-verified no accuracy loss). The
host ships stu pre-transposed [30, 512] so a single ACT sigmoid yields
the sel strip directly — no PE transposes, identity, or strip copy.
l1 = (2 R-pairs + sel + 4 U-pairs) x 2 m-tiles = 14 matmuls.
"""

import numpy as np

B = 4096
NCORES = 8
BL = B // NCORES          # 512 local batch
KN = 30
NOUT = 1024
NT = NOUT // 128          # 8 exercise tiles
P = 128
S_N = 100000
N_WARM = 3

_PROG_CACHE = {}


def _np_f16():
    import ml_dtypes
    return np.dtype(ml_dtypes.bfloat16)


def _np_f8():
    import ml_dtypes
    return np.dtype(ml_dtypes.float8_e4m3)


def _host_prep(q_idx, fm_vars, w1, b1, w2, b2, w3, b3):
    """Pair tables + folded W1F + packed weight layouts (all host-side)."""
    f16 = _np_f16()
    f8 = _np_f8()
    q = np.asarray(q_idx).astype(np.int64)            # [1024, 3] sorted asc
    fm = np.asarray(fm_vars, dtype=np.float64)
    w1 = np.asarray(w1, np.float64)

    chi = np.abs(fm)
    f0, f1, f3 = chi[0], chi[1], chi[3]
    F0 = np.minimum(f0, 1.0)
    F1 = np.minimum(f1, 1.0)
    F2 = np.minimum(np.maximum(f0, f1) + chi[2], 1.0)
    F3 = np.minimum(f3, 1.0)
    F4 = np.minimum(np.maximum(f3, f0) + chi[4], 1.0)
    F5 = np.minimum(np.maximum(f3, f1) + chi[5], 1.0)
    m0, m1, m3 = F0, F1, F3
    m2 = F2 - F0 - F1
    m4 = F4 - F0 - F3
    m5 = F5 - F1 - F3
    m6 = 1.0 - F2 - F4 - F5 + F0 + F1 + F3
    # C = c0 x0 + c1 x1 + c2 x2 + a01 r01 + a02 r02 + a12 r12 + aU min(r02,r12)
    c0 = m0 + m2 + m4 + m6
    c1 = m1 + m5
    c2 = m3
    a01 = -(m2 + m6)
    a02 = -m4
    a12 = -m5
    aU = -m6

    # pair table (ordered pairs i<j as they appear; q columns sorted asc)
    pairs = {}

    def pid(i, j):
        key = (int(i), int(j))
        if key not in pairs:
            pairs[key] = len(pairs)
        return pairs[key]

    p01 = np.array([pid(q[n, 0], q[n, 1]) for n in range(NOUT)])
    p02 = np.array([pid(q[n, 0], q[n, 2]) for n in range(NOUT)])
    p12 = np.array([pid(q[n, 1], q[n, 2]) for n in range(NOUT)])
    NP = len(pairs)                                    # ~435
    PI = np.empty(NP, np.int64)
    PJ = np.empty(NP, np.int64)
    for (i, j), p in pairs.items():
        PI[p], PJ[p] = i, j

    # gp: pair strip table. tile s holds pairs [128s..128s+cols) at cols
    # s*128. [30, 4*128] fp16.
    n_ptile = (NP + P - 1) // P                        # 4
    assert n_ptile == 4 and NP - 3 * P <= 51 + 20
    gp = np.zeros((KN, 4 * P), np.float32)
    for p in range(NP):
        s, c = p // P, p % P
        gp[PI[p], s * P + c] += 1.0
        gp[PJ[p], s * P + c] -= 1.0

    # gu: per-exercise-tile E columns. slot idx = 2t+pl (pl 0->d02, 1->d12)
    # occupies cols idx*128. [30, 16*128] fp16.
    gu = np.zeros((KN, 16 * P), np.float32)
    for t in range(NT):
        for pl in range(2):
            idx = 2 * t + pl
            nn = np.arange(t * P, (t + 1) * P)
            src = q[nn, 0] if pl == 0 else q[nn, 1]
            gu[src, idx * P + (nn % P)] += 1.0
            gu[q[nn, 2], idx * P + (nn % P)] -= 1.0

    # W1F fold: features order = [R(0..NP-1); sel(30); U(1024)]
    KF_R = NP                                          # 435
    W1F = np.zeros((256, KF_R + KN + NOUT), np.float64)
    np.add.at(W1F.T, p01, (a01 * w1).T)
    np.add.at(W1F.T, p02, (a02 * w1).T)
    np.add.at(W1F.T, p12, (a12 * w1).T)
    for k, c in enumerate((c0, c1, c2)):
        np.add.at(W1F.T, KF_R + q[:, k], (c * w1).T)
    W1F[:, KF_R + KN:] = aU * w1

    # bf16 sel chunk [128, 2*128]: w1fa[k, m*128+mc] = W1F[m*128+mc, NP+k]
    w1fa = np.zeros((P, 2 * P), np.float32)
    w1fa[0:KN, 0:P] = W1F[0:P, NP:NP + KN].T
    w1fa[0:KN, P:2 * P] = W1F[P:2 * P, NP:NP + KN].T
    w1fa = w1fa.astype(f16)

    # fp8 DoubleRow packing: slot idx = m*12 + 2j+i, j=0,1 -> R pairs
    # (R0,R1), (R2,Rrem); j=2..5 -> U pairs. w1f8[k, idx, mc].
    WF = np.zeros((256, 4 * P + NT * P))               # R padded to 512 + U
    WF[:, 0:NP] = W1F[:, 0:NP]
    WF[:, 4 * P:] = W1F[:, NP + KN:]
    w1f8 = np.zeros((P, 24, P), np.float64)
    for m in range(2):
        for j in range(6):
            for i in range(2):
                ch = 2 * j + i                          # chunk in WF
                w1f8[:, m * 12 + 2 * j + i, :] = \
                    WF[m * P:(m + 1) * P, ch * P:(ch + 1) * P].T
    w1f8 = np.ascontiguousarray(w1f8.reshape(P, 24 * P)).astype(f8)

    w2t = np.asarray(w2, np.float32).T.reshape(2, P, P)     # [k, p, o]
    w2s = np.ascontiguousarray(
        w2t.transpose(1, 0, 2).reshape(P, 2 * P)).astype(f16)
    w3s = np.ascontiguousarray(np.asarray(w3, np.float32).T).astype(f16)
    b1c = np.ascontiguousarray(np.asarray(b1, np.float32).reshape(2, P).T)
    b2c = np.ascontiguousarray(np.asarray(b2, np.float32).reshape(1, P).T)
    b3c = np.ascontiguousarray(np.asarray(b3, np.float32).reshape(NT, P).T)

    # packb [128, 779] f32: w2s(128) | b1(2) | b2(1) | b3(8) |
    # w1fa(128, bf16 pairs) | w3(512, bf16 pairs)
    packb = np.zeros((P, 779), np.float32)
    packb[:, 0:128] = w2s.view(np.float32)
    packb[:, 128:130] = b1c
    packb[:, 130:131] = b2c
    packb[:, 131:139] = b3c
    packb[:, 139:267] = w1fa.view(np.float32)
    packb[:, 267:779] = w3s.view(np.float32)

    # pack2 [30, 1280] f32: gp(256) | gu(1024)   (bf16 pairs as f32 words)
    pack2 = np.zeros((KN, 1280), np.float32)
    pack2[:, 0:256] = gp.astype(f16).view(np.float32)
    pack2[:, 256:1280] = gu.astype(f16).view(np.float32)

    return dict(pack2=pack2, packb=packb, w1f8=w1f8)


def _build_program():
    key = "v9"
    if key in _PROG_CACHE:
        return _PROG_CACHE[key]

    import concourse.bacc as bacc
    import concourse.mybir as mybir
    import concourse.tile as tile

    f32 = mybir.dt.float32
    f16 = mybir.dt.bfloat16
    f8 = mybir.dt.float8e4
    AF = mybir.ActivationFunctionType
    ALU = mybir.AluOpType
    DR = mybir.MatmulPerfMode.DoubleRow

    nc = bacc.Bacc("TRN2", target_bir_lowering=False, debug=False,
                   num_swdge_queues=4)

    stu_d = nc.dram_tensor("stuT", [KN, BL], f32,
                           kind="ExternalInput").ap()
    pack2_d = nc.dram_tensor("pack2", [KN, 1280], f32,
                             kind="ExternalInput").ap()
    packb_d = nc.dram_tensor("packb", [P, 779], f32,
                             kind="ExternalInput").ap()
    w1f8_d = nc.dram_tensor("w1f8", [P, 24 * P], f8,
                            kind="ExternalInput").ap()
    out_d = nc.dram_tensor("out", [P, NT * (BL // 2)], f32,
                           kind="ExternalOutput").ap()

    def mm(out, lhsT, rhs, start, stop, tile_position=None, perf_mode=None):
        nc.tensor.matmul(out, lhsT, rhs, start=start, stop=stop,
                         tile_position=tile_position, perf_mode=perf_mode)

    with tile.TileContext(nc) as tc:
        with (
            tc.tile_pool(name="const", bufs=1) as cpool,
            tc.tile_pool(name="work", bufs=4) as wpool,
            tc.tile_pool(name="pgen", bufs=6, space="PSUM") as pgen,
            tc.tile_pool(name="pl1", bufs=2, space="PSUM") as pl1,
        ):
            # ---- input DMAs: stuT halves on sync+scalar (critical path);
            # weights in need-order on the gpsimd queue ----
            # ACT table preload first so the scalar dispatch doesn't delay it
            dum = cpool.tile([P, 2], f32, tag="dum")
            nc.vector.memset(dum[:, 0:1], 0.0)
            nc.scalar.activation(dum[:, 1:2], dum[:, 0:1], AF.Sigmoid)

            stuT_s = cpool.tile([KN, BL], f32, tag="stuT")
            nc.sync.dma_start(stuT_s[:, 0:BL // 2], stu_d[:, 0:BL // 2])
            nc.scalar.dma_start(stuT_s[:, BL // 2:], stu_d[:, BL // 2:])
            pack2_s = cpool.tile([KN, 1280], f32, tag="pack2")
            nc.gpsimd.dma_start(pack2_s[:], pack2_d[:])
            w1f8_s = cpool.tile([P, 24, P], f8, tag="w1f8")
            nc.gpsimd.dma_start(w1f8_s[:, :, :], w1f8_d[:])
            packb_s = cpool.tile([P, 779], f32, tag="packb")
            nc.gpsimd.dma_start(packb_s[:], packb_d[:])

            w2v = packb_s[:, 0:128].bitcast(f16)           # [128, 256]
            b1v = packb_s[:, 128:130]
            b2v = packb_s[:, 130:131]
            b3v = packb_s[:, 131:139]
            w1fa_s = packb_s[:, 139:267].bitcast(f16)      # [128, 256]
            w3_s = packb_s[:, 267:779].bitcast(f16)        # [128, 1024]
            gpv = pack2_s[:, 0:256].bitcast(f16)           # [30, 512]
            guv = pack2_s[:, 256:1280].bitcast(f16)        # [30, 2048]

            # ---- PE warm-up ----
            warm = cpool.tile([32, BL], f16, tag="warm")
            nc.vector.memset(warm[:], 0.0)
            wps = pgen.tile([P, BL], f32, tag="g", name="wps")
            for _ in range(N_WARM):
                mm(wps[0:32, :], warm[0:32, 0:32], warm[0:32, :],
                   True, True, tile_position=(0, 0))

            osb_big = cpool.tile([P, NT * BL], f16, tag="osb_big")

            # ---- sel strip: sigmoid halves straight from stuT ----
            selS = cpool.tile([KN, BL], f16, tag="selS")
            nc.scalar.activation(selS[:, 0:BL // 2], stuT_s[:, 0:BL // 2],
                                 AF.Sigmoid)
            nc.scalar.activation(selS[:, BL // 2:], stuT_s[:, BL // 2:],
                                 AF.Sigmoid)

            # ---- shared tiles ----
            # fp8 DoubleRow rhs pairs: R01=(R0,R1), R2r=(R2,Rrem), U pairs
            R01 = cpool.tile([P, 2, BL], f8, tag="R01")
            R2r = cpool.tile([P, 2, BL], f8, tag="R2r")
            # zero-pad Rrem plane rows 32:128 (relu later fills 0:51)
            nc.vector.memset(R2r[32:64, 1, :], 0.0)
            nc.vector.memset(R2r[64:128, 1, :], 0.0)
            U2 = [cpool.tile([P, 2, BL], f8, tag=f"U2_{j}", name=f"U2_{j}")
                  for j in range(4)]
            h1 = cpool.tile([P, 2 * BL], f16, tag="h1")
            h2 = cpool.tile([P, BL], f16, tag="h2")
            l1ps = {}

            def front_full():
                # pairs: 4 strip matmuls, one full bank each
                dps = []
                for s in range(4):
                    cols = P if s < 3 else 51
                    dp = pgen.tile([P, BL], f32, tag="g", name=f"dp{s}")
                    mm(dp[0:cols, :], gpv[0:KN, s * P:s * P + cols],
                       selS[:, :], True, True, tile_position=(0, 0))
                    dps.append(dp)
                # R relus: R0/R1 on DVE, R2 + remainder on ACT (fp8 out)
                nc.vector.tensor_scalar(R01[:, 0, :], dps[0][:],
                                        0.0, None, ALU.max)
                nc.vector.tensor_scalar(R01[:, 1, :], dps[1][:],
                                        0.0, None, ALU.max)
                nc.scalar.activation(R2r[:, 0, :], dps[2][:], AF.Relu)
                nc.scalar.activation(R2r[0:51, 1, :], dps[3][0:51, :],
                                     AF.Relu)

            def l1_dr(j, rhs_tile, start, stop):
                for m in range(2):
                    if m not in l1ps:
                        l1ps[m] = pl1.tile([P, BL], f32, tag="l1",
                                           name=f"l1_{m}")
                    mm(l1ps[m][:, :],
                       w1f8_s[:, m * 12 + 2 * j:m * 12 + 2 * j + 2, :],
                       rhs_tile[:, :, :], start, stop, perf_mode=DR)

            def l1_sel():
                for m in range(2):
                    mm(l1ps[m][:, :], w1fa_s[0:KN, m * P:(m + 1) * P],
                       selS[:, :], False, False)

            def u_tile(t):
                eb = []
                for pl in range(2):
                    idx = 2 * t + pl
                    ep = pgen.tile([P, BL], f32, tag="g", name=f"e{t}{pl}")
                    mm(ep[:], guv[0:KN, idx * P:(idx + 1) * P],
                       selS[:, :], True, True, tile_position=(0, 0))
                    eb.append(ep)
                # r02 = relu(E02) on ACT; U = (E12 max 0) min r02 on DVE
                r02 = wpool.tile([P, BL], f16, tag="r02")
                nc.scalar.activation(r02[:], eb[0][:], AF.Relu)
                nc.vector.scalar_tensor_tensor(
                    U2[t // 2][:, t % 2, :], eb[1][:], 0.0, r02[:],
                    ALU.max, ALU.min)

            def mlp_head():
                # relu(z1 + b1): DVE for m0, ACT for m1
                nc.vector.tensor_scalar(h1[:, 0:BL], l1ps[0][:, :],
                                        b1v[:, 0:1], 0.0, ALU.add, ALU.max)
                nc.scalar.activation(h1[:, BL:2 * BL], l1ps[1][:, :],
                                     AF.Relu, bias=b1v[:, 1:2])

            def mlp_l2():
                l2p = pgen.tile([P, BL], f32, tag="g", name="l2")
                mm(l2p[:], w2v[:, 0:P], h1[:, 0:BL], True, False)
                mm(l2p[:], w2v[:, P:2 * P], h1[:, BL:2 * BL], False, True)
                nc.vector.tensor_scalar(h2[:], l2p[:], b2v[:, 0:1], 0.0,
                                        ALU.add, ALU.max)

            def mlp_l3(o):
                bank = pgen.tile([P, BL], f32, tag="g", name=f"l3_{o}")
                mm(bank[:], w3_s[:, o * P:(o + 1) * P], h2[:], True, True)
                nc.scalar.activation(
                    osb_big[:, o * BL:(o + 1) * BL],
                    bank[:], AF.Sigmoid, bias=b3v[:, o:o + 1])

            def out_dma(o):
                eng = nc.sync if o % 2 == 0 else nc.gpsimd
                eng.dma_start(
                    out_d[:, o * (BL // 2):(o + 1) * (BL // 2)],
                    osb_big[:, o * BL:(o + 1) * BL].bitcast(f32))

            # ---------------- schedule ----------------
            front_full()
            u_tile(0)
            u_tile(1)
            l1_dr(0, R01, True, False)
            u_tile(2)
            l1_dr(1, R2r, False, False)
            u_tile(3)
            l1_sel()
            u_tile(4)
            u_tile(5)
            l1_dr(2, U2[0], False, False)
            u_tile(6)
            l1_dr(3, U2[1], False, False)
            u_tile(7)
            l1_dr(4, U2[2], False, False)
            l1_dr(5, U2[3], False, True)
            mlp_head()
            mlp_l2()
            for o in range(NT):
                mlp_l3(o)
                out_dma(o)

    nc.compile()
    _PROG_CACHE[key] = nc
    return nc


def _run(inputs, trace=False, tmpdir=None, **_kw):
    from concourse import bass_utils

    nc = _build_program()

    prep = _host_prep(inputs["q_idx"], inputs["fm_vars"],
                      inputs["w1"], inputs["b1"], inputs["w2"], inputs["b2"],
                      inputs["w3"], inputs["b3"])
    emb = np.asarray(inputs["emb"], np.float32)
    stu_id = np.asarray(inputs["stu_id"]).astype(np.int64)

    in_maps = []
    for c in range(NCORES):
        rows = emb[stu_id[c * BL:(c + 1) * BL]]              # [512, 30]
        stuT = np.ascontiguousarray(rows.T).astype(np.float32)
        in_maps.append(dict(stuT=stuT, **prep))

    if trace:
        import sys, types
        if "antenv.axon_hooks" not in sys.modules:
            import trn_agent_boot.trn_boot as tb
            mod = types.ModuleType("antenv.axon_hooks")
            hook = tb._ntff_profile_via_ctypes("/opt/axon/libaxon_pjrt.so")
            mod.get_axon_ntff_profile_hook = lambda: hook
            mod.set_axon_ntff_profile_hook = lambda h: None
            sys.modules["antenv.axon_hooks"] = mod
        bass_utils.upload_artifacts = lambda d: d

    res = bass_utils.run_bass_kernel_spmd(
        nc, in_maps, core_ids=list(range(NCORES)), trace=trace, tmpdir=tmpdir)

    outs = []
    for c in range(NCORES):
        arr = np.ascontiguousarray(res.results[c]["out"]).view(_np_f16())
        arr = arr.reshape(P, NT, BL)              # [p, o, b]
        arr = arr.transpose(2, 1, 0).reshape(BL, NOUT)      # [b, n]
        outs.append(arr)
    out = np.concatenate(outs, axis=0)
    return np.ascontiguousarray(out.astype(np.float32)), res


def kernel(**inputs):
    out, _ = _run(inputs, trace=False)
    return out


# revision 19
# speedup vs baseline: 1.0600x; 1.0600x over previous
"""Trainium2 Bass kernel for nn_CICDM — pair-feature reformulation, v7.

Math: the Choquet integral C[n,b] is linear in shared features
  F = [R (435 pair hinges), sel (30), U (1024 per-exercise triple mins)]
  R[p=(i<j)] = relu(sel_i - sel_j)
  U[n] = min(R[p02(n)], R[p12(n)]) = relu(min(d02, d12))
so layer-1 of the MLP folds the whole per-exercise coefficient structure
into a host-precomputed W1F = w1 @ Gamma^T:  z1 = W1F @ F + b1.
The device never materializes C.

v7: R and U feature blocks both run as fp8-e4m3 DoubleRow matmuls (only
the 30-row sel chunk stays bf16; host-verified no accuracy loss). The
host ships stu pre-transposed [30, 512] so a single ACT sigmoid yields
the sel strip directly — no PE transposes, identity, or strip copy.
l1 = (2 R-pairs + sel + 4 U-pairs) x 2 m-tiles = 14 matmuls.
"""

import numpy as np

B = 4096
NCORES = 8
BL = B // NCORES          # 512 local batch
KN = 30
NOUT = 1024
NT = NOUT // 128          # 8 exercise tiles
P = 128
S_N = 100000
N_WARM = 3

_PROG_CACHE = {}


def _np_f16():
    import ml_dtypes
    return np.dtype(ml_dtypes.bfloat16)


def _np_f8():
    import ml_dtypes
    return np.dtype(ml_dtypes.float8_e4m3)


def _host_prep(q_idx, fm_vars, w1, b1, w2, b2, w3, b3):
    """Pair tables + folded W1F + packed weight layouts (all host-side)."""
    f16 = _np_f16()
    f8 = _np_f8()
    q = np.asarray(q_idx).astype(np.int64)            # [1024, 3] sorted asc
    fm = np.asarray(fm_vars, dtype=np.float64)
    w1 = np.asarray(w1, np.float64)

    chi = np.abs(fm)
    f0, f1, f3 = chi[0], chi[1], chi[3]
    F0 = np.minimum(f0, 1.0)
    F1 = np.minimum(f1, 1.0)
    F2 = np.minimum(np.maximum(f0, f1) + chi[2], 1.0)
    F3 = np.minimum(f3, 1.0)
    F4 = np.minimum(np.maximum(f3, f0) + chi[4], 1.0)
    F5 = np.minimum(np.maximum(f3, f1) + chi[5], 1.0)
    m0, m1, m3 = F0, F1, F3
    m2 = F2 - F0 - F1
    m4 = F4 - F0 - F3
    m5 = F5 - F1 - F3
    m6 = 1.0 - F2 - F4 - F5 + F0 + F1 + F3
    # C = c0 x0 + c1 x1 + c2 x2 + a01 r01 + a02 r02 + a12 r12 + aU min(r02,r12)
    c0 = m0 + m2 + m4 + m6
    c1 = m1 + m5
    c2 = m3
    a01 = -(m2 + m6)
    a02 = -m4
    a12 = -m5
    aU = -m6

    # pair table (ordered pairs i<j as they appear; q columns sorted asc)
    pairs = {}

    def pid(i, j):
        key = (int(i), int(j))
        if key not in pairs:
            pairs[key] = len(pairs)
        return pairs[key]

    p01 = np.array([pid(q[n, 0], q[n, 1]) for n in range(NOUT)])
    p02 = np.array([pid(q[n, 0], q[n, 2]) for n in range(NOUT)])
    p12 = np.array([pid(q[n, 1], q[n, 2]) for n in range(NOUT)])
    NP = len(pairs)                                    # ~435
    PI = np.empty(NP, np.int64)
    PJ = np.empty(NP, np.int64)
    for (i, j), p in pairs.items():
        PI[p], PJ[p] = i, j

    # gp: pair strip table. tile s holds pairs [128s..128s+cols) at cols
    # s*128. [30, 4*128] fp16. (odd slots run from a row-64 device copy)
    n_ptile = (NP + P - 1) // P                        # 4
    assert n_ptile == 4 and NP - 3 * P <= 51 + 20
    gp = np.zeros((KN, 4 * P), np.float32)
    for p in range(NP):
        s, c = p // P, p % P
        gp[PI[p], s * P + c] += 1.0
        gp[PJ[p], s * P + c] -= 1.0

    # gu: per-exercise-tile E columns. slot idx = 2t+pl (pl 0->d02,
    # 1->d12) at cols idx*128. [30, 16*128] fp16.
    gu = np.zeros((KN, 16 * P), np.float32)
    for t in range(NT):
        for pl in range(2):
            idx = 2 * t + pl
            nn = np.arange(t * P, (t + 1) * P)
            src = q[nn, 0] if pl == 0 else q[nn, 1]
            gu[src, idx * P + (nn % P)] += 1.0
            gu[q[nn, 2], idx * P + (nn % P)] -= 1.0

    # W1F fold: features order = [R(0..NP-1); sel(30); U(1024)]
    KF_R = NP                                          # 435
    W1F = np.zeros((256, KF_R + KN + NOUT), np.float64)
    np.add.at(W1F.T, p01, (a01 * w1).T)
    np.add.at(W1F.T, p02, (a02 * w1).T)
    np.add.at(W1F.T, p12, (a12 * w1).T)
    for k, c in enumerate((c0, c1, c2)):
        np.add.at(W1F.T, KF_R + q[:, k], (c * w1).T)
    W1F[:, KF_R + KN:] = aU * w1

    # bf16 sel chunk [128, 2*128]: w1fa[k, m*128+mc] = W1F[m*128+mc, NP+k]
    w1fa = np.zeros((P, 2 * P), np.float32)
    w1fa[0:KN, 0:P] = W1F[0:P, NP:NP + KN].T
    w1fa[0:KN, P:2 * P] = W1F[P:2 * P, NP:NP + KN].T
    w1fa = w1fa.astype(f16)

    # fp8 DoubleRow packing: slot idx = m*12 + 2j+i, j=0,1 -> R pairs
    # (R0,R1), (R2,Rrem); j=2..5 -> U pairs. w1f8[k, idx, mc].
    WF = np.zeros((256, 4 * P + NT * P))               # R padded to 512 + U
    WF[:, 0:NP] = W1F[:, 0:NP]
    WF[:, 4 * P:] = W1F[:, NP + KN:]
    w1f8 = np.zeros((P, 24, P), np.float64)
    for m in range(2):
        for j in range(6):
            for i in range(2):
                ch = 2 * j + i                          # chunk in WF
                w1f8[:, m * 12 + 2 * j + i, :] = \
                    WF[m * P:(m + 1) * P, ch * P:(ch + 1) * P].T
    w1f8 = np.ascontiguousarray(w1f8.reshape(P, 24 * P)).astype(f8)

    w2t = np.asarray(w2, np.float32).T.reshape(2, P, P)     # [k, p, o]
    w2s = np.ascontiguousarray(
        w2t.transpose(1, 0, 2).reshape(P, 2 * P)).astype(f16)
    w3s = np.ascontiguousarray(np.asarray(w3, np.float32).T).astype(f16)
    b1c = np.ascontiguousarray(np.asarray(b1, np.float32).reshape(2, P).T)
    b2c = np.ascontiguousarray(np.asarray(b2, np.float32).reshape(1, P).T)
    b3c = np.ascontiguousarray(np.asarray(b3, np.float32).reshape(NT, P).T)

    # packb [128, 779] f32: w2s(128) | b1(2) | b2(1) | b3(8) |
    # w1fa(128, bf16 pairs) | w3(512, bf16 pairs)
    packb = np.zeros((P, 779), np.float32)
    packb[:, 0:128] = w2s.view(np.float32)
    packb[:, 128:130] = b1c
    packb[:, 130:131] = b2c
    packb[:, 131:139] = b3c
    packb[:, 139:267] = w1fa.view(np.float32)
    packb[:, 267:779] = w3s.view(np.float32)

    # pack2 [30, 1280] f32: gp(256) | gu(1024)   (bf16 pairs as f32 words)
    pack2 = np.zeros((KN, 1280), np.float32)
    pack2[:, 0:256] = gp.astype(f16).view(np.float32)
    pack2[:, 256:1280] = gu.astype(f16).view(np.float32)

    return dict(pack2=pack2, packb=packb, w1f8=w1f8)


def _build_program():
    key = "v11"
    if key in _PROG_CACHE:
        return _PROG_CACHE[key]

    import concourse.bacc as bacc
    import concourse.mybir as mybir
    import concourse.tile as tile

    f32 = mybir.dt.float32
    f16 = mybir.dt.bfloat16
    f8 = mybir.dt.float8e4
    AF = mybir.ActivationFunctionType
    ALU = mybir.AluOpType
    DR = mybir.MatmulPerfMode.DoubleRow

    nc = bacc.Bacc("TRN2", target_bir_lowering=False, debug=False,
                   num_swdge_queues=4)

    stu_d = nc.dram_tensor("stuT", [KN, BL], f32,
                           kind="ExternalInput").ap()
    pack2_d = nc.dram_tensor("pack2", [KN, 1280], f32,
                             kind="ExternalInput").ap()
    packb_d = nc.dram_tensor("packb", [P, 779], f32,
                             kind="ExternalInput").ap()
    w1f8_d = nc.dram_tensor("w1f8", [P, 24 * P], f8,
                            kind="ExternalInput").ap()
    out_d = nc.dram_tensor("out", [P, NT * (BL // 2)], f32,
                           kind="ExternalOutput").ap()

    def mm(out, lhsT, rhs, start, stop, tile_position=None, perf_mode=None):
        nc.tensor.matmul(out, lhsT, rhs, start=start, stop=stop,
                         tile_position=tile_position, perf_mode=perf_mode)

    with tile.TileContext(nc) as tc:
        with (
            tc.tile_pool(name="const", bufs=1) as cpool,
            tc.tile_pool(name="work", bufs=4) as wpool,
            tc.tile_pool(name="pgen", bufs=6, space="PSUM") as pgen,
            tc.tile_pool(name="pl1", bufs=2, space="PSUM") as pl1,
        ):
            # ---- input DMAs: stuT halves on sync+scalar (critical path);
            # weights in need-order on the gpsimd queue ----
            # ACT table preload first so the scalar dispatch doesn't delay it
            dum = cpool.tile([P, 2], f32, tag="dum")
            nc.vector.memset(dum[:, 0:1], 0.0)
            nc.scalar.activation(dum[:, 1:2], dum[:, 0:1], AF.Sigmoid)

            stuT_s = cpool.tile([KN, BL], f32, tag="stuT")
            nc.sync.dma_start(stuT_s[:, 0:BL // 2], stu_d[:, 0:BL // 2])
            nc.scalar.dma_start(stuT_s[:, BL // 2:], stu_d[:, BL // 2:])
            pack2_s = cpool.tile([94, 1280], f32, tag="pack2")
            nc.gpsimd.dma_start(pack2_s[0:KN, :], pack2_d[:])
            w1f8_s = cpool.tile([P, 24, P], f8, tag="w1f8")
            nc.gpsimd.dma_start(w1f8_s[:, :, :], w1f8_d[:])
            packb_s = cpool.tile([P, 779], f32, tag="packb")
            nc.gpsimd.dma_start(packb_s[:], packb_d[:])

            w2v = packb_s[:, 0:128].bitcast(f16)           # [128, 256]
            b1v = packb_s[:, 128:130]
            b2v = packb_s[:, 130:131]
            b3v = packb_s[:, 131:139]
            w1fa_s = packb_s[:, 139:267].bitcast(f16)      # [128, 256]
            w3_s = packb_s[:, 267:779].bitcast(f16)        # [128, 1024]
            gpv = pack2_s[:, 0:256].bitcast(f16)           # [94, 512]
            guv = pack2_s[:, 256:1280].bitcast(f16)        # [94, 2048]
            p2all = pack2_s[:, :].bitcast(f16)             # [94, 2560]
            # row-64 copies for the second PE row group (DVE, 16-bit 2x)
            nc.vector.tensor_copy(p2all[64:64 + KN, :], p2all[0:KN, :])

            # ---- PE warm-up ----
            warm = cpool.tile([32, BL], f16, tag="warm")
            nc.vector.memset(warm[:], 0.0)
            wps = pgen.tile([P, BL], f32, tag="g", name="wps")
            for _ in range(N_WARM):
                mm(wps[0:32, :], warm[0:32, 0:32], warm[0:32, :],
                   True, True, tile_position=(0, 0))

            osb_big = cpool.tile([P, NT * BL], f16, tag="osb_big")

            # ---- sel strip: sigmoid halves straight from stuT ----
            selS = cpool.tile([94, BL], f16, tag="selS")
            nc.scalar.activation(selS[0:KN, 0:BL // 2],
                                 stuT_s[:, 0:BL // 2], AF.Sigmoid)
            nc.scalar.activation(selS[0:KN, BL // 2:],
                                 stuT_s[:, BL // 2:], AF.Sigmoid)
            nc.vector.tensor_copy(selS[64:64 + KN, :], selS[0:KN, :])

            # ---- shared tiles ----
            # fp8 DoubleRow rhs pairs: R01=(R0,R1), R2r=(R2,Rrem), U pairs
            R01 = cpool.tile([P, 2, BL], f8, tag="R01")
            R2r = cpool.tile([P, 2, BL], f8, tag="R2r")
            # zero-pad Rrem plane rows 32:128 (relu later fills 0:51)
            nc.vector.memset(R2r[32:64, 1, :], 0.0)
            nc.vector.memset(R2r[64:128, 1, :], 0.0)
            U2 = [cpool.tile([P, 2, BL], f8, tag=f"U2_{j}", name=f"U2_{j}")
                  for j in range(4)]
            h1 = cpool.tile([P, 2 * BL], f16, tag="h1")
            h2 = cpool.tile([P, BL], f16, tag="h2")
            l1ps = {}

            def front_full():
                # pairs: 4 strip matmuls, one full bank each
                dps = []
                for s in range(4):
                    cols = P if s < 3 else 51
                    ro = 64 * (s % 2)
                    dp = pgen.tile([P, BL], f32, tag="g", name=f"dp{s}")
                    mm(dp[0:cols, :], gpv[ro:ro + KN, s * P:s * P + cols],
                       selS[ro:ro + KN, :], True, True,
                       tile_position=(ro, 0))
                    dps.append(dp)
                # R relus: R0/R1 on DVE, R2 + remainder on ACT (fp8 out)
                nc.vector.tensor_scalar(R01[:, 0, :], dps[0][:],
                                        0.0, None, ALU.max)
                nc.vector.tensor_scalar(R01[:, 1, :], dps[1][:],
                                        0.0, None, ALU.max)
                nc.scalar.activation(R2r[:, 0, :], dps[2][:], AF.Relu)
                nc.scalar.activation(R2r[0:51, 1, :], dps[3][0:51, :],
                                     AF.Relu)

            def l1_dr(j, rhs_tile, start, stop):
                for m in range(2):
                    if m not in l1ps:
                        l1ps[m] = pl1.tile([P, BL], f32, tag="l1",
                                           name=f"l1_{m}")
                    mm(l1ps[m][:, :],
                       w1f8_s[:, m * 12 + 2 * j:m * 12 + 2 * j + 2, :],
                       rhs_tile[:, :, :], start, stop, perf_mode=DR)

            def l1_sel():
                for m in range(2):
                    mm(l1ps[m][:, :], w1fa_s[0:KN, m * P:(m + 1) * P],
                       selS[0:KN, :], False, False)

            def u_tile(t):
                eb = []
                for pl in range(2):
                    idx = 2 * t + pl
                    ro = 64 * (idx % 2)
                    ep = pgen.tile([P, BL], f32, tag="g", name=f"e{t}{pl}")
                    mm(ep[:], guv[ro:ro + KN, idx * P:(idx + 1) * P],
                       selS[ro:ro + KN, :], True, True,
                       tile_position=(ro, 0))
                    eb.append(ep)
                # r02 = relu(E02) on ACT; U = (E12 max 0) min r02 on DVE
                r02 = wpool.tile([P, BL], f16, tag="r02")
                nc.scalar.activation(r02[:], eb[0][:], AF.Relu)
                nc.vector.scalar_tensor_tensor(
                    U2[t // 2][:, t % 2, :], eb[1][:], 0.0, r02[:],
                    ALU.max, ALU.min)

            def mlp_head():
                # relu(z1 + b1): DVE for m0, ACT for m1
                nc.vector.tensor_scalar(h1[:, 0:BL], l1ps[0][:, :],
                                        b1v[:, 0:1], 0.0, ALU.add, ALU.max)
                nc.scalar.activation(h1[:, BL:2 * BL], l1ps[1][:, :],
                                     AF.Relu, bias=b1v[:, 1:2])

            def mlp_l2():
                l2p = pgen.tile([P, BL], f32, tag="g", name="l2")
                mm(l2p[:], w2v[:, 0:P], h1[:, 0:BL], True, False)
                mm(l2p[:], w2v[:, P:2 * P], h1[:, BL:2 * BL], False, True)
                nc.vector.tensor_scalar(h2[:], l2p[:], b2v[:, 0:1], 0.0,
                                        ALU.add, ALU.max)

            def mlp_l3(o):
                bank = pgen.tile([P, BL], f32, tag="g", name=f"l3_{o}")
                mm(bank[:], w3_s[:, o * P:(o + 1) * P], h2[:], True, True)
                nc.scalar.activation(
                    osb_big[:, o * BL:(o + 1) * BL],
                    bank[:], AF.Sigmoid, bias=b3v[:, o:o + 1])

            def out_dma(o):
                nc.sync.dma_start(
                    out_d[:, o * (BL // 2):(o + 1) * (BL // 2)],
                    osb_big[:, o * BL:(o + 1) * BL].bitcast(f32))

            # ---------------- schedule ----------------
            front_full()
            u_tile(0)
            u_tile(1)
            l1_dr(0, R01, True, False)
            u_tile(2)
            l1_dr(1, R2r, False, False)
            u_tile(3)
            l1_sel()
            u_tile(4)
            u_tile(5)
            l1_dr(2, U2[0], False, False)
            u_tile(6)
            l1_dr(3, U2[1], False, False)
            u_tile(7)
            l1_dr(4, U2[2], False, False)
            l1_dr(5, U2[3], False, True)
            mlp_head()
            mlp_l2()
            for o in range(NT):
                mlp_l3(o)
                out_dma(o)

    nc.compile()
    _PROG_CACHE[key] = nc
    return nc


def _run(inputs, trace=False, tmpdir=None, **_kw):
    from concourse import bass_utils

    nc = _build_program()

    prep = _host_prep(inputs["q_idx"], inputs["fm_vars"],
                      inputs["w1"], inputs["b1"], inputs["w2"], inputs["b2"],
                      inputs["w3"], inputs["b3"])
    emb = np.asarray(inputs["emb"], np.float32)
    stu_id = np.asarray(inputs["stu_id"]).astype(np.int64)

    in_maps = []
    for c in range(NCORES):
        rows = emb[stu_id[c * BL:(c + 1) * BL]]              # [512, 30]
        stuT = np.ascontiguousarray(rows.T).astype(np.float32)
        in_maps.append(dict(stuT=stuT, **prep))

    if trace:
        import sys, types
        if "antenv.axon_hooks" not in sys.modules:
            import trn_agent_boot.trn_boot as tb
            mod = types.ModuleType("antenv.axon_hooks")
            hook = tb._ntff_profile_via_ctypes("/opt/axon/libaxon_pjrt.so")
            mod.get_axon_ntff_profile_hook = lambda: hook
            mod.set_axon_ntff_profile_hook = lambda h: None
            sys.modules["antenv.axon_hooks"] = mod
        bass_utils.upload_artifacts = lambda d: d

    res = bass_utils.run_bass_kernel_spmd(
        nc, in_maps, core_ids=list(range(NCORES)), trace=trace, tmpdir=tmpdir)

    outs = []
    for c in range(NCORES):
        arr = np.ascontiguousarray(res.results[c]["out"]).view(_np_f16())
        arr = arr.reshape(P, NT, BL)              # [p, o, b]
        arr = arr.transpose(2, 1, 0).reshape(BL, NOUT)      # [b, n]
        outs.append(arr)
    out = np.concatenate(outs, axis=0)
    return np.ascontiguousarray(out.astype(np.float32)), res


def kernel(**inputs):
    out, _ = _run(inputs, trace=False)
    return out


# revision 20
# speedup vs baseline: 1.0661x; 1.0057x over previous
"""Trainium2 Bass kernel for nn_CICDM — pair-feature reformulation, v7.

Math: the Choquet integral C[n,b] is linear in shared features
  F = [R (435 pair hinges), sel (30), U (1024 per-exercise triple mins)]
  R[p=(i<j)] = relu(sel_i - sel_j)
  U[n] = min(R[p02(n)], R[p12(n)]) = relu(min(d02, d12))
so layer-1 of the MLP folds the whole per-exercise coefficient structure
into a host-precomputed W1F = w1 @ Gamma^T:  z1 = W1F @ F + b1.
The device never materializes C.

v7: R and U feature blocks both run as fp8-e4m3 DoubleRow matmuls (only
the 30-row sel chunk stays bf16; host-verified no accuracy loss). The
host ships stu pre-transposed [30, 512] so a single ACT sigmoid yields
the sel strip directly — no PE transposes, identity, or strip copy.
l1 = (2 R-pairs + sel + 4 U-pairs) x 2 m-tiles = 14 matmuls.
"""

import numpy as np

B = 4096
NCORES = 8
BL = B // NCORES          # 512 local batch
KN = 30
NOUT = 1024
NT = NOUT // 128          # 8 exercise tiles
P = 128
S_N = 100000
N_WARM = 5

_PROG_CACHE = {}


def _np_f16():
    import ml_dtypes
    return np.dtype(ml_dtypes.bfloat16)


def _np_f8():
    import ml_dtypes
    return np.dtype(ml_dtypes.float8_e4m3)


def _host_prep(q_idx, fm_vars, w1, b1, w2, b2, w3, b3):
    """Pair tables + folded W1F + packed weight layouts (all host-side)."""
    f16 = _np_f16()
    f8 = _np_f8()
    q = np.asarray(q_idx).astype(np.int64)            # [1024, 3] sorted asc
    fm = np.asarray(fm_vars, dtype=np.float64)
    w1 = np.asarray(w1, np.float64)

    chi = np.abs(fm)
    f0, f1, f3 = chi[0], chi[1], chi[3]
    F0 = np.minimum(f0, 1.0)
    F1 = np.minimum(f1, 1.0)
    F2 = np.minimum(np.maximum(f0, f1) + chi[2], 1.0)
    F3 = np.minimum(f3, 1.0)
    F4 = np.minimum(np.maximum(f3, f0) + chi[4], 1.0)
    F5 = np.minimum(np.maximum(f3, f1) + chi[5], 1.0)
    m0, m1, m3 = F0, F1, F3
    m2 = F2 - F0 - F1
    m4 = F4 - F0 - F3
    m5 = F5 - F1 - F3
    m6 = 1.0 - F2 - F4 - F5 + F0 + F1 + F3
    # C = c0 x0 + c1 x1 + c2 x2 + a01 r01 + a02 r02 + a12 r12 + aU min(r02,r12)
    c0 = m0 + m2 + m4 + m6
    c1 = m1 + m5
    c2 = m3
    a01 = -(m2 + m6)
    a02 = -m4
    a12 = -m5
    aU = -m6

    # pair table (ordered pairs i<j as they appear; q columns sorted asc)
    pairs = {}

    def pid(i, j):
        key = (int(i), int(j))
        if key not in pairs:
            pairs[key] = len(pairs)
        return pairs[key]

    p01 = np.array([pid(q[n, 0], q[n, 1]) for n in range(NOUT)])
    p02 = np.array([pid(q[n, 0], q[n, 2]) for n in range(NOUT)])
    p12 = np.array([pid(q[n, 1], q[n, 2]) for n in range(NOUT)])
    NP = len(pairs)                                    # ~435
    PI = np.empty(NP, np.int64)
    PJ = np.empty(NP, np.int64)
    for (i, j), p in pairs.items():
        PI[p], PJ[p] = i, j

    # gp: pair strip table. tile s holds pairs [128s..128s+cols) at cols
    # s*128. [30, 4*128] fp16. (odd slots run from a row-64 device copy)
    n_ptile = (NP + P - 1) // P                        # 4
    assert n_ptile == 4 and NP - 3 * P <= 51 + 20
    gp = np.zeros((KN, 4 * P), np.float32)
    for p in range(NP):
        s, c = p // P, p % P
        gp[PI[p], s * P + c] += 1.0
        gp[PJ[p], s * P + c] -= 1.0

    # gu: per-exercise-tile E columns. slot idx = 2t+pl (pl 0->d02,
    # 1->d12) at cols idx*128. [30, 16*128] fp16.
    gu = np.zeros((KN, 16 * P), np.float32)
    for t in range(NT):
        for pl in range(2):
            idx = 2 * t + pl
            nn = np.arange(t * P, (t + 1) * P)
            src = q[nn, 0] if pl == 0 else q[nn, 1]
            gu[src, idx * P + (nn % P)] += 1.0
            gu[q[nn, 2], idx * P + (nn % P)] -= 1.0

    # W1F fold: features order = [R(0..NP-1); sel(30); U(1024)]
    KF_R = NP                                          # 435
    W1F = np.zeros((256, KF_R + KN + NOUT), np.float64)
    np.add.at(W1F.T, p01, (a01 * w1).T)
    np.add.at(W1F.T, p02, (a02 * w1).T)
    np.add.at(W1F.T, p12, (a12 * w1).T)
    for k, c in enumerate((c0, c1, c2)):
        np.add.at(W1F.T, KF_R + q[:, k], (c * w1).T)
    W1F[:, KF_R + KN:] = aU * w1

    # bf16 sel chunk [128, 2*128]: w1fa[k, m*128+mc] = W1F[m*128+mc, NP+k]
    w1fa = np.zeros((P, 2 * P), np.float32)
    w1fa[0:KN, 0:P] = W1F[0:P, NP:NP + KN].T
    w1fa[0:KN, P:2 * P] = W1F[P:2 * P, NP:NP + KN].T
    w1fa = w1fa.astype(f16)

    # fp8 DoubleRow packing: slot idx = m*12 + 2j+i, j=0,1 -> R pairs
    # (R0,R1), (R2,Rrem); j=2..5 -> U pairs. w1f8[k, idx, mc].
    WF = np.zeros((256, 4 * P + NT * P))               # R padded to 512 + U
    WF[:, 0:NP] = W1F[:, 0:NP]
    WF[:, 4 * P:] = W1F[:, NP + KN:]
    w1f8 = np.zeros((P, 24, P), np.float64)
    for m in range(2):
        for j in range(6):
            for i in range(2):
                ch = 2 * j + i                          # chunk in WF
                w1f8[:, m * 12 + 2 * j + i, :] = \
                    WF[m * P:(m + 1) * P, ch * P:(ch + 1) * P].T
    w1f8 = np.ascontiguousarray(w1f8.reshape(P, 24 * P)).astype(f8)

    w2t = np.asarray(w2, np.float32).T.reshape(2, P, P)     # [k, p, o]
    w2s = np.ascontiguousarray(
        w2t.transpose(1, 0, 2).reshape(P, 2 * P)).astype(f16)
    w3s = np.ascontiguousarray(np.asarray(w3, np.float32).T).astype(f16)
    b1c = np.ascontiguousarray(np.asarray(b1, np.float32).reshape(2, P).T)
    b2c = np.ascontiguousarray(np.asarray(b2, np.float32).reshape(1, P).T)
    b3c = np.ascontiguousarray(np.asarray(b3, np.float32).reshape(NT, P).T)

    # packb [128, 779] f32: w2s(128) | b1(2) | b2(1) | b3(8) |
    # w1fa(128, bf16 pairs) | w3(512, bf16 pairs)
    packb = np.zeros((P, 779), np.float32)
    packb[:, 0:128] = w2s.view(np.float32)
    packb[:, 128:130] = b1c
    packb[:, 130:131] = b2c
    packb[:, 131:139] = b3c
    packb[:, 139:267] = w1fa.view(np.float32)
    packb[:, 267:779] = w3s.view(np.float32)

    # pack2 [30, 1280] f32: gp(256) | gu(1024)   (bf16 pairs as f32 words)
    pack2 = np.zeros((KN, 1280), np.float32)
    pack2[:, 0:256] = gp.astype(f16).view(np.float32)
    pack2[:, 256:1280] = gu.astype(f16).view(np.float32)

    return dict(pack2=pack2, packb=packb, w1f8=w1f8)


def _build_program():
    key = "v12"
    if key in _PROG_CACHE:
        return _PROG_CACHE[key]

    import concourse.bacc as bacc
    import concourse.mybir as mybir
    import concourse.tile as tile

    f32 = mybir.dt.float32
    f16 = mybir.dt.bfloat16
    f8 = mybir.dt.float8e4
    AF = mybir.ActivationFunctionType
    ALU = mybir.AluOpType
    DR = mybir.MatmulPerfMode.DoubleRow

    nc = bacc.Bacc("TRN2", target_bir_lowering=False, debug=False,
                   num_swdge_queues=4)

    stu_d = nc.dram_tensor("stuT", [KN, BL], f32,
                           kind="ExternalInput").ap()
    pack2_d = nc.dram_tensor("pack2", [KN, 1280], f32,
                             kind="ExternalInput").ap()
    packb_d = nc.dram_tensor("packb", [P, 779], f32,
                             kind="ExternalInput").ap()
    w1f8_d = nc.dram_tensor("w1f8", [P, 24 * P], f8,
                            kind="ExternalInput").ap()
    out_d = nc.dram_tensor("out", [P, NT * (BL // 2)], f32,
                           kind="ExternalOutput").ap()

    def mm(out, lhsT, rhs, start, stop, tile_position=None, perf_mode=None):
        nc.tensor.matmul(out, lhsT, rhs, start=start, stop=stop,
                         tile_position=tile_position, perf_mode=perf_mode)

    with tile.TileContext(nc) as tc:
        with (
            tc.tile_pool(name="const", bufs=1) as cpool,
            tc.tile_pool(name="work", bufs=4) as wpool,
            tc.tile_pool(name="pgen", bufs=6, space="PSUM") as pgen,
            tc.tile_pool(name="pl1", bufs=2, space="PSUM") as pl1,
        ):
            # ---- input DMAs: stuT halves on sync+scalar (critical path);
            # weights in need-order on the gpsimd queue ----
            # ACT table preload first so the scalar dispatch doesn't delay it
            dum = cpool.tile([P, 2], f32, tag="dum")
            nc.vector.memset(dum[:, 0:1], 0.0)
            nc.scalar.activation(dum[:, 1:2], dum[:, 0:1], AF.Sigmoid)

            stuT_s = cpool.tile([KN, BL], f32, tag="stuT")
            nc.sync.dma_start(stuT_s[:, 0:BL // 2], stu_d[:, 0:BL // 2])
            nc.scalar.dma_start(stuT_s[:, BL // 2:], stu_d[:, BL // 2:])
            pack2_s = cpool.tile([94, 1280], f32, tag="pack2")
            nc.gpsimd.dma_start(pack2_s[0:KN, :], pack2_d[:])
            w1f8_s = cpool.tile([P, 24, P], f8, tag="w1f8")
            nc.gpsimd.dma_start(w1f8_s[:, :, :], w1f8_d[:])
            packb_s = cpool.tile([P, 779], f32, tag="packb")
            nc.gpsimd.dma_start(packb_s[:], packb_d[:])

            w2v = packb_s[:, 0:128].bitcast(f16)           # [128, 256]
            b1v = packb_s[:, 128:130]
            b2v = packb_s[:, 130:131]
            b3v = packb_s[:, 131:139]
            w1fa_s = packb_s[:, 139:267].bitcast(f16)      # [128, 256]
            w3_s = packb_s[:, 267:779].bitcast(f16)        # [128, 1024]
            gpv = pack2_s[:, 0:256].bitcast(f16)           # [94, 512]
            guv = pack2_s[:, 256:1280].bitcast(f16)        # [94, 2048]
            p2all = pack2_s[:, :].bitcast(f16)             # [94, 2560]
            # row-64 copies for the second PE row group (DVE, 16-bit 2x);
            # gp part first so front's q64 strip starts sooner
            nc.vector.tensor_copy(p2all[64:64 + KN, 0:512],
                                  p2all[0:KN, 0:512])
            nc.vector.tensor_copy(p2all[64:64 + KN, 512:],
                                  p2all[0:KN, 512:])

            # ---- PE warm-up ----
            warm = cpool.tile([32, BL], f16, tag="warm")
            nc.vector.memset(warm[:], 0.0)
            wps = pgen.tile([P, BL], f32, tag="g", name="wps")
            for _ in range(N_WARM):
                mm(wps[0:32, :], warm[0:32, 0:32], warm[0:32, :],
                   True, True, tile_position=(0, 0))

            osb_big = cpool.tile([P, NT * BL], f16, tag="osb_big")

            # ---- sel strip: sigmoid halves straight from stuT ----
            selS = cpool.tile([94, BL], f16, tag="selS")
            nc.scalar.activation(selS[0:KN, 0:BL // 2],
                                 stuT_s[:, 0:BL // 2], AF.Sigmoid)
            nc.scalar.activation(selS[0:KN, BL // 2:],
                                 stuT_s[:, BL // 2:], AF.Sigmoid)
            nc.vector.tensor_copy(selS[64:64 + KN, :], selS[0:KN, :])

            # ---- shared tiles ----
            # fp8 DoubleRow rhs pairs: R01=(R0,R1), R2r=(R2,Rrem), U pairs
            R01 = cpool.tile([P, 2, BL], f8, tag="R01")
            R2r = cpool.tile([P, 2, BL], f8, tag="R2r")
            # zero-pad Rrem plane rows 32:128 (relu later fills 0:51)
            nc.vector.memset(R2r[32:64, 1, :], 0.0)
            nc.vector.memset(R2r[64:128, 1, :], 0.0)
            U2 = [cpool.tile([P, 2, BL], f8, tag=f"U2_{j}", name=f"U2_{j}")
                  for j in range(4)]
            h1 = cpool.tile([P, 2 * BL], f16, tag="h1")
            h2 = cpool.tile([P, BL], f16, tag="h2")
            l1ps = {}

            def front_full():
                # pairs: 4 strip matmuls, one full bank each
                dps = []
                for s in range(4):
                    cols = P if s < 3 else 51
                    ro = 64 * (s % 2)
                    dp = pgen.tile([P, BL], f32, tag="g", name=f"dp{s}")
                    mm(dp[0:cols, :], gpv[ro:ro + KN, s * P:s * P + cols],
                       selS[ro:ro + KN, :], True, True,
                       tile_position=(ro, 0))
                    dps.append(dp)
                # R relus: R0/R1 on DVE, R2 + remainder on ACT (fp8 out)
                nc.vector.tensor_scalar(R01[:, 0, :], dps[0][:],
                                        0.0, None, ALU.max)
                nc.vector.tensor_scalar(R01[:, 1, :], dps[1][:],
                                        0.0, None, ALU.max)
                nc.scalar.activation(R2r[:, 0, :], dps[2][:], AF.Relu)
                nc.scalar.activation(R2r[0:51, 1, :], dps[3][0:51, :],
                                     AF.Relu)

            def l1_dr(j, rhs_tile, start, stop):
                for m in range(2):
                    if m not in l1ps:
                        l1ps[m] = pl1.tile([P, BL], f32, tag="l1",
                                           name=f"l1_{m}")
                    mm(l1ps[m][:, :],
                       w1f8_s[:, m * 12 + 2 * j:m * 12 + 2 * j + 2, :],
                       rhs_tile[:, :, :], start, stop, perf_mode=DR)

            def l1_sel():
                for m in range(2):
                    mm(l1ps[m][:, :], w1fa_s[0:KN, m * P:(m + 1) * P],
                       selS[0:KN, :], False, False)

            def u_tile(t):
                eb = []
                for pl in range(2):
                    idx = 2 * t + pl
                    ro = 64 * (idx % 2)
                    ep = pgen.tile([P, BL], f32, tag="g", name=f"e{t}{pl}")
                    mm(ep[:], guv[ro:ro + KN, idx * P:(idx + 1) * P],
                       selS[ro:ro + KN, :], True, True,
                       tile_position=(ro, 0))
                    eb.append(ep)
                # r02 = relu(E02) on ACT; U = (E12 max 0) min r02 on DVE
                r02 = wpool.tile([P, BL], f16, tag="r02")
                nc.scalar.activation(r02[:], eb[0][:], AF.Relu)
                nc.vector.scalar_tensor_tensor(
                    U2[t // 2][:, t % 2, :], eb[1][:], 0.0, r02[:],
                    ALU.max, ALU.min)

            def mlp_head():
                # relu(z1 + b1): DVE for m0, ACT for m1
                nc.vector.tensor_scalar(h1[:, 0:BL], l1ps[0][:, :],
                                        b1v[:, 0:1], 0.0, ALU.add, ALU.max)
                nc.scalar.activation(h1[:, BL:2 * BL], l1ps[1][:, :],
                                     AF.Relu, bias=b1v[:, 1:2])

            def mlp_l2():
                l2p = pgen.tile([P, BL], f32, tag="g", name="l2")
                mm(l2p[:], w2v[:, 0:P], h1[:, 0:BL], True, False)
                mm(l2p[:], w2v[:, P:2 * P], h1[:, BL:2 * BL], False, True)
                nc.vector.tensor_scalar(h2[:], l2p[:], b2v[:, 0:1], 0.0,
                                        ALU.add, ALU.max)

            def mlp_l3(o):
                bank = pgen.tile([P, BL], f32, tag="g", name=f"l3_{o}")
                mm(bank[:], w3_s[:, o * P:(o + 1) * P], h2[:], True, True)
                nc.scalar.activation(
                    osb_big[:, o * BL:(o + 1) * BL],
                    bank[:], AF.Sigmoid, bias=b3v[:, o:o + 1])

            def out_dma(o):
                nc.sync.dma_start(
                    out_d[:, o * (BL // 2):(o + 1) * (BL // 2)],
                    osb_big[:, o * BL:(o + 1) * BL].bitcast(f32))

            # ---------------- schedule ----------------
            front_full()
            u_tile(0)
            u_tile(1)
            l1_dr(0, R01, True, False)
            u_tile(2)
            l1_dr(1, R2r, False, False)
            u_tile(3)
            l1_sel()
            u_tile(4)
            u_tile(5)
            l1_dr(2, U2[0], False, False)
            u_tile(6)
            l1_dr(3, U2[1], False, False)
            u_tile(7)
            l1_dr(4, U2[2], False, False)
            l1_dr(5, U2[3], False, True)
            mlp_head()
            mlp_l2()
            for o in range(NT):
                mlp_l3(o)
                out_dma(o)

    nc.compile()
    _PROG_CACHE[key] = nc
    return nc


def _run(inputs, trace=False, tmpdir=None, **_kw):
    from concourse import bass_utils

    nc = _build_program()

    prep = _host_prep(inputs["q_idx"], inputs["fm_vars"],
                      inputs["w1"], inputs["b1"], inputs["w2"], inputs["b2"],
                      inputs["w3"], inputs["b3"])
    emb = np.asarray(inputs["emb"], np.float32)
    stu_id = np.asarray(inputs["stu_id"]).astype(np.int64)

    in_maps = []
    for c in range(NCORES):
        rows = emb[stu_id[c * BL:(c + 1) * BL]]              # [512, 30]
        stuT = np.ascontiguousarray(rows.T).astype(np.float32)
        in_maps.append(dict(stuT=stuT, **prep))

    if trace:
        import sys, types
        if "antenv.axon_hooks" not in sys.modules:
            import trn_agent_boot.trn_boot as tb
            mod = types.ModuleType("antenv.axon_hooks")
            hook = tb._ntff_profile_via_ctypes("/opt/axon/libaxon_pjrt.so")
            mod.get_axon_ntff_profile_hook = lambda: hook
            mod.set_axon_ntff_profile_hook = lambda h: None
            sys.modules["antenv.axon_hooks"] = mod
        bass_utils.upload_artifacts = lambda d: d

    res = bass_utils.run_bass_kernel_spmd(
        nc, in_maps, core_ids=list(range(NCORES)), trace=trace, tmpdir=tmpdir)

    outs = []
    for c in range(NCORES):
        arr = np.ascontiguousarray(res.results[c]["out"]).view(_np_f16())
        arr = arr.reshape(P, NT, BL)              # [p, o, b]
        arr = arr.transpose(2, 1, 0).reshape(BL, NOUT)      # [b, n]
        outs.append(arr)
    out = np.concatenate(outs, axis=0)
    return np.ascontiguousarray(out.astype(np.float32)), res


def kernel(**inputs):
    out, _ = _run(inputs, trace=False)
    return out
